# revision 23
# baseline (speedup 1.0000x reference)
"""GroupEmbedding Trainium2 kernel v5: token-major + TensorE reduction.

v4.x spent ~1.4ms of DVE on the select/count-multiply + (m,h)-tree. v5 moves
the whole token->user reduction to the idle Tensor engine:

- Item tokens are gathered USER-MAJOR (token i = user*20 + m at partition
  i%128, row i//128). A window of 20 blocks (2560 tokens) covers exactly 128
  users, so a constant one-hot matrix W_k[t, f] = [f == (128k+t)//20]
  (k = block-within-window, 20 patterns, data-independent) turns the m-sum
  into 20 accumulating matmuls per window: psum[user, (h,d)] += W_k.T @ X.
- The quad-slot select folds into a per-token prescale (count * onehot4,
  host-built mask, one DVE mult per chunk) and a tiny per-window h-fold of
  the psum (2 adds).
- The group-sum (50 users -> group) is another constant-pattern matmul over
  partitions: Wseg_k[p, g] = [g == (128k+p)//50], 25 patterns, 50 matmuls
  per wave accumulating into a [64-group] psum (two halves per wave).
- The user/sim path also runs user-major; per-user target-sim rows come from
  a separate sim-quad table (512B descriptors).

DVE keeps only: prescale mults, psum h-folds, user/target quad-selects, the
sim dot, and the personalization mults  (~0.8ms -> DMA becomes the pacer).
"""
from contextlib import ExitStack

import numpy as np

import concourse.bass as bass
import concourse.bacc as bacc
import concourse.mybir as mybir
import concourse.tile as tile
from concourse.bass_utils import run_bass_kernel_spmd

G, U, M = 4096, 50, 20
D = 64
V = 100000
Q = V // 4                   # 25000 quads (int16-addressable)
FACTOR = 0.5
NCORES = 8
GPC = G // NCORES            # 512 groups per core
NWAVE = GPC // 128           # 4 waves of 128 groups
UW = 128 * U                 # 6400 users per wave
TW = UW * M                  # 128000 item tokens per wave
NBLK = TW // 128             # 1000 blocks per wave
WINB = 20                    # blocks per 128-user window
NWIN = NBLK // WINB          # 50 windows per wave
CHB = 8                      # blocks per item gather call (1024 idxs)
NCHUNK = NBLK // CHB         # 125 chunks per wave
UCALLS = (8, 8, 8, 8, 8, 8, 2)   # user gather call sizes (rows)
UCOLS = sum(8 * n for n in UCALLS)  # 400 idx cols

f32 = mybir.dt.float32
bf16 = mybir.dt.bfloat16
i16 = mybir.dt.int16

_CACHE = {}


def _ensure_ntff_hook():
    try:
        import antenv.axon_hooks  # noqa: F401
        return
    except ImportError:
        pass
    import contextlib
    import ctypes
    import sys
    import types

    mod = types.ModuleType("antenv.axon_hooks")
    holder = {}
    mod.set_axon_ntff_profile_hook = lambda h: holder.__setitem__("h", h)
    mod.get_axon_ntff_profile_hook = lambda: holder.get("h")
    try:
        lib = ctypes.CDLL("/opt/axon/libaxon_pjrt.so")
        if hasattr(lib, "axon_start_nrt_profile"):
            lib.axon_start_nrt_profile.argtypes = [
                ctypes.POINTER(ctypes.c_int64), ctypes.c_size_t]
            lib.axon_start_nrt_profile.restype = ctypes.c_int64
            lib.axon_stop_nrt_profile.argtypes = [ctypes.c_char_p]
            lib.axon_stop_nrt_profile.restype = ctypes.c_int64

            @contextlib.contextmanager
            def _hook(output_dir, device_ids):
                import jax
                jax.devices()
                if device_ids:
                    ids = (ctypes.c_int64 * len(device_ids))(*device_ids)
                    rc = lib.axon_start_nrt_profile(ids, len(device_ids))
                else:
                    rc = lib.axon_start_nrt_profile(None, 0)
                if rc != 0:
                    raise RuntimeError(f"axon_start_nrt_profile rc={rc}")
                try:
                    yield
                finally:
                    n = lib.axon_stop_nrt_profile(str(output_dir).encode())
                    print(f"ntff profile: {n} file(s) -> {output_dir}",
                          file=sys.stderr)

            holder["h"] = _hook
    except OSError:
        pass
    import antenv
    sys.modules["antenv.axon_hooks"] = mod
    antenv.axon_hooks = mod


def _build_program():
    nc = bacc.Bacc("TRN2", target_bir_lowering=False, debug=False,
                   num_devices=NCORES, dynamic_dma_scratch_size=1 << 15,
                   num_swdge_queues=4)
    itemq = nc.dram_tensor("itemq", [Q, 4 * D], bf16,
                           kind="ExternalInput").ap()
    usimq = nc.dram_tensor("usimq", [Q, 8 * D], bf16,
                           kind="ExternalInput").ap()
    simq = nc.dram_tensor("simq", [Q, 4 * D], bf16,
                          kind="ExternalInput").ap()
    item_i16 = nc.dram_tensor("item_i16", [NWAVE, 128, NCHUNK * 64], i16,
                              kind="ExternalInput").ap()
    s8d = nc.dram_tensor("s8", [NWAVE, 128, NBLK, 8], bf16,
                         kind="ExternalInput").ap()
    u_i16 = nc.dram_tensor("u_i16", [NWAVE, 128, UCOLS], i16,
                           kind="ExternalInput").ap()
    t_i16 = nc.dram_tensor("t_i16", [NWAVE, 128, UCOLS], i16,
                           kind="ExternalInput").ap()
    um8d = nc.dram_tensor("um8", [NWAVE, 128, U, 8], bf16,
                          kind="ExternalInput").ap()
    tm8d = nc.dram_tensor("tm8", [NWAVE, 128, U, 8], bf16,
                          kind="ExternalInput").ap()
    witemd = nc.dram_tensor("witem", [128, WINB * 128], bf16,
                            kind="ExternalInput").ap()
    wsegd = nc.dram_tensor("wseg", [128, 25 * 64], bf16,
                           kind="ExternalInput").ap()
    out = nc.dram_tensor("out", [GPC, D], f32, kind="ExternalOutput").ap()

    mult = mybir.AluOpType.mult
    add = mybir.AluOpType.add

    def tt(eng, o, a, b, op):
        eng.tensor_tensor(out=o, in0=a, in1=b, op=op)

    qctr = [0]

    def nextq():
        q = qctr[0] % 4
        qctr[0] += 1
        return q

    with tile.TileContext(nc) as tc:
        with ExitStack() as ctx:
            p_w = ctx.enter_context(tc.tile_pool(name="w", bufs=1))
            p_wi = ctx.enter_context(tc.tile_pool(name="wi", bufs=2))
            p_s8 = ctx.enter_context(tc.tile_pool(name="s8", bufs=1))
            p_gq = ctx.enter_context(tc.tile_pool(name="gq", bufs=9))
            p_uq = ctx.enter_context(tc.tile_pool(name="uq", bufs=1))
            p_ub = ctx.enter_context(tc.tile_pool(name="ub", bufs=1))
            p_sm = ctx.enter_context(tc.tile_pool(name="sm", bufs=1))
            p_ps = ctx.enter_context(tc.psum_pool(name="ps", bufs=6))
            p_pg = ctx.enter_context(tc.psum_pool(name="pg", bufs=2))

            wt = p_w.tile([128, WINB, 128], bf16, tag="wt")
            nc.sync.dma_start(wt[:], witemd[:])
            ws = p_w.tile([128, 25, 64], bf16, tag="ws")
            nc.sync.dma_start(ws[:], wsegd[:])

            for w in range(NWAVE):
                ixt = p_wi.tile([128, NCHUNK * 64], i16, tag="ix")
                nc.sync.dma_start(ixt[:], item_i16[w])
                s8w = p_s8.tile([128, NBLK, 8], bf16, tag="s8")
                nc.sync.dma_start(s8w[:], s8d[w])
                uxt = p_wi.tile([128, UCOLS], i16, tag="ux")
                nc.sync.dma_start(uxt[:], u_i16[w])
                txt = p_wi.tile([128, UCOLS], i16, tag="tx")
                nc.sync.dma_start(txt[:], t_i16[w])
                umt = p_wi.tile([128, U, 8], bf16, tag="um")
                nc.sync.dma_start(umt[:], um8d[w])
                tmt = p_wi.tile([128, U, 8], bf16, tag="tm")
                nc.sync.dma_start(tmt[:], tm8d[w])

                uq = p_uq.tile([128, U, 8 * D], bf16, tag="uq")
                tq = p_uq.tile([128, U, 4 * D], bf16, tag="tq")
                # user/target gather calls, spread through the chunk stream
                ucall = []
                r0 = 0
                for j, nrow in enumerate(UCALLS):
                    ucall.append(("u", r0, nrow, 8 * r0))
                    ucall.append(("t", r0, nrow, 8 * r0))
                    r0 += nrow
                uspread = {30 + 6 * j: c for j, c in enumerate(ucall)}

                ubw = p_ub.tile([128, NWIN, D], f32)
                cur_ps = None
                for c in range(NCHUNK):
                    gq = p_gq.tile([128, CHB, 4 * D], bf16)
                    nc.gpsimd.dma_gather(
                        out_ap=gq[:], in_ap=itemq[:],
                        idxs_ap=ixt[:, c * 64:(c + 1) * 64],
                        num_idxs=1024, num_idxs_reg=1024,
                        elem_size=4 * D, queue_num=nextq())
                    # prescale: count * onehot4 (dup pairs for 32-bit reads)
                    gv = gq[:].rearrange("p b (h s t) -> p (b h) s t",
                                         h=4, t=2)
                    sv = s8w[:, c * CHB:(c + 1) * CHB].rearrange(
                        "p b (h t) -> p (b h) t", h=4).unsqueeze(
                        2).to_broadcast([128, CHB * 4, D // 2, 2])
                    tt(nc.vector, gv, gv, sv, mult)
                    for b in range(CHB):
                        bg = c * CHB + b
                        k = bg % WINB
                        if k == 0:
                            cur_ps = p_ps.tile([128, 4 * D], f32, tag="ps")
                        nc.tensor.matmul(
                            cur_ps[:], wt[:, k, :], gq[:, b, :],
                            start=(k == 0), stop=(k == WINB - 1))
                        if k == WINB - 1:
                            win = bg // WINB
                            pv = cur_ps[:].rearrange("p (h d) -> p d h", h=4)
                            nc.vector.reduce_sum(
                                out=ubw[:, win, :], in_=pv,
                                axis=mybir.AxisListType.X)
                    if c in uspread:
                        kind, r0, nrow, col = uspread[c]
                        if kind == "u":
                            nc.gpsimd.dma_gather(
                                out_ap=uq[:, r0:r0 + nrow, :], in_ap=usimq[:],
                                idxs_ap=uxt[:, col:col + 8 * nrow],
                                num_idxs=nrow * 128, num_idxs_reg=nrow * 128,
                                elem_size=8 * D, queue_num=nextq())
                        else:
                            nc.gpsimd.dma_gather(
                                out_ap=tq[:, r0:r0 + nrow, :], in_ap=simq[:],
                                idxs_ap=txt[:, col:col + 8 * nrow],
                                num_idxs=nrow * 128, num_idxs_reg=nrow * 128,
                                elem_size=4 * D, queue_num=nextq())

                # ---- epilogue (user-major) ----
                uv = uq[:].rearrange("p u (h s t) -> p (u h) s t", h=4, t=2)
                umv = umt[:].rearrange("p u (h t) -> p (u h) t",
                                       h=4).unsqueeze(2).to_broadcast(
                    [128, U * 4, D, 2])
                tt(nc.vector, uv, uv, umv, mult)
                hu = uq[:].rearrange("p u (h e) -> p u h e", h=4)
                tt(nc.vector, hu[:, :, 0:2, :], hu[:, :, 0:2, :],
                   hu[:, :, 2:4, :], add)
                tt(nc.vector, hu[:, :, 0:1, :], hu[:, :, 0:1, :],
                   hu[:, :, 1:2, :], add)
                us = hu[:, :, 0, :]                    # [128, U, 2D]
                tv = tq[:].rearrange("p u (h s t) -> p (u h) s t", h=4, t=2)
                tmv = tmt[:].rearrange("p u (h t) -> p (u h) t",
                                       h=4).unsqueeze(2).to_broadcast(
                    [128, U * 4, D // 2, 2])
                tt(nc.vector, tv, tv, tmv, mult)
                ht = tq[:].rearrange("p u (h e) -> p u h e", h=4)
                tt(nc.vector, ht[:, :, 0:2, :], ht[:, :, 0:2, :],
                   ht[:, :, 2:4, :], add)
                tt(nc.vector, ht[:, :, 0:1, :], ht[:, :, 0:1, :],
                   ht[:, :, 1:2, :], add)
                ts = ht[:, :, 0, :]                    # [128, U, D]
                sg = us[:, :, D:2 * D]
                tt(nc.vector, sg, sg, ts, mult)
                simw = p_sm.tile([128, U], f32, tag="sw")
                nc.vector.reduce_sum(out=simw[:], in_=sg,
                                     axis=mybir.AxisListType.X)
                simw2 = p_sm.tile([128, U], bf16, tag="sw2")
                nc.vector.tensor_scalar_mul(out=simw2[:], in0=simw[:],
                                            scalar1=FACTOR)
                pers = p_sm.tile([128, U, D], bf16, tag="pers")
                tt(nc.vector, pers[:], ubw[:], us[:, :, 0:D], mult)
                tt(nc.vector, pers[:], pers[:],
                   simw2[:].unsqueeze(2).to_broadcast([128, U, D]), mult)
                # group-sum: 50 seg-matmuls into two 64-group psums
                for hf in range(2):
                    pg = p_pg.tile([128, D], f32, tag="pg")
                    for k in range(25):
                        nc.tensor.matmul(
                            pg[0:64, :], ws[:, k, 0:64], pers[:, hf * 25 + k, :],
                            start=(k == 0), stop=(k == 24))
                    rr = p_sm.tile([128, D], f32, tag=f"r{hf}")
                    nc.vector.tensor_copy(out=rr[0:64, :], in_=pg[0:64, :])
                    nc.sync.dma_start(
                        out[w * 128 + hf * 64:w * 128 + (hf + 1) * 64, :],
                        rr[0:64, :])
    nc.finalize()
    return nc


def _wrap16(q):
    """[..., 128, N] slot-ordered (slot i = j*128 + p) -> 16-wrapped tiles."""
    lead = q.shape[:-2]
    n = q.shape[-1]
    ni16 = 128 * n // 16
    f = np.swapaxes(q, -1, -2).reshape(lead + (128 * n,))
    w = np.swapaxes(f.reshape(lead + (ni16, 16)), -1, -2)
    w = np.broadcast_to(w[..., None, :, :], lead + (8, 16, ni16))
    return np.ascontiguousarray(w.reshape(lead + (128, ni16)))


def _wrap_calls(vals, sizes):
    """vals [..., Ntot] position-ordered -> concat of per-call wrap16 tiles."""
    outs = []
    base = 0
    for n in sizes:
        sl = vals[..., base:base + n * 128].reshape(vals.shape[:-1] + (n, 128))
        sl = np.swapaxes(sl, -1, -2)           # [..., 128, n]
        outs.append(_wrap16(sl))
        base += n * 128
    return np.ascontiguousarray(np.concatenate(outs, axis=-1))


def _onehot8(h, val):
    """one-hot over 4 with value `val`, duplicated pairs -> [..., 8] bf16."""
    import ml_dtypes
    oh = np.zeros(h.shape + (4,), dtype=np.float32)
    np.put_along_axis(oh, h[..., None].astype(np.int64), val[..., None], -1)
    oh = oh.astype(ml_dtypes.bfloat16)
    return np.ascontiguousarray(
        np.stack([oh, oh], axis=-1).reshape(h.shape + (8,)))


def _prep_in_maps(group_user, behavior_ids, behavior_counts, target_user,
                  similarity_vec, user_emb_w, item_emb_w):
    import ml_dtypes
    bf = ml_dtypes.bfloat16

    itemq = np.ascontiguousarray(item_emb_w, dtype=np.float32).astype(
        bf).reshape(Q, 4 * D)
    usimq = np.concatenate(
        [np.asarray(user_emb_w, np.float32),
         np.asarray(similarity_vec, np.float32)], axis=1).astype(bf).reshape(
        Q, 8 * D)
    simq = np.ascontiguousarray(similarity_vec, dtype=np.float32).astype(
        bf).reshape(Q, 4 * D)

    # item tokens, user-major: token i = u_wave*20 + m
    ids = np.asarray(behavior_ids, np.int64).reshape(NCORES, NWAVE, TW)
    cc = np.asarray(behavior_counts, np.float32).reshape(NCORES, NWAVE, TW)
    item_i16 = _wrap_calls((ids // 4).astype(np.int16), [CHB] * NCHUNK)
    # s8: [K, W, NBLK, 128, 8] -> [K, W, 128, NBLK, 8]
    h = (ids % 4).reshape(NCORES, NWAVE, NBLK, 128)
    cb = cc.reshape(NCORES, NWAVE, NBLK, 128)
    s8 = _onehot8(h, cb).transpose(0, 1, 3, 2, 4)

    gu = np.asarray(group_user, np.int64).reshape(NCORES, NWAVE, UW)
    tu = np.asarray(target_user, np.int64).reshape(NCORES, NWAVE, 128)
    tu_rep = np.repeat(tu, U, axis=2)          # user-major replication
    u_i16 = _wrap_calls((gu // 4).astype(np.int16), list(UCALLS))
    t_i16 = _wrap_calls((tu_rep // 4).astype(np.int16), list(UCALLS))
    um8 = _onehot8((gu % 4).reshape(NCORES, NWAVE, U, 128),
                   np.ones((NCORES, NWAVE, U, 128), np.float32)).transpose(
        0, 1, 3, 2, 4)
    tm8 = _onehot8((tu_rep % 4).reshape(NCORES, NWAVE, U, 128),
                   np.ones((NCORES, NWAVE, U, 128), np.float32)).transpose(
        0, 1, 3, 2, 4)

    # constant one-hot matrices
    t_idx = np.arange(128)
    witem = np.zeros((128, WINB, 128), np.float32)
    for k in range(WINB):
        witem[t_idx, k, (128 * k + t_idx) // M] = 1.0
    wseg = np.zeros((128, 25, 64), np.float32)
    for k in range(25):
        wseg[t_idx, k, (128 * k + t_idx) // U] = 1.0
    witem = witem.astype(bf).reshape(128, WINB * 128)
    wseg = wseg.astype(bf).reshape(128, 25 * 64)

    in_maps = []
    for k in range(NCORES):
        in_maps.append({
            "itemq": itemq,
            "usimq": usimq,
            "simq": simq,
            "item_i16": np.ascontiguousarray(item_i16[k]),
            "s8": np.ascontiguousarray(s8[k]),
            "u_i16": np.ascontiguousarray(u_i16[k]),
            "t_i16": np.ascontiguousarray(t_i16[k]),
            "um8": np.ascontiguousarray(um8[k]),
            "tm8": np.ascontiguousarray(tm8[k]),
            "witem": witem,
            "wseg": wseg,
        })
    return in_maps


def kernel(group_user, behavior_ids, behavior_counts, target_user,
           similarity_vec, user_emb_w, item_emb_w, _trace=False):
    _ensure_ntff_hook()
    if "nc" not in _CACHE:
        _CACHE["nc"] = _build_program()
    nc = _CACHE["nc"]
    in_maps = _prep_in_maps(group_user, behavior_ids, behavior_counts,
                            target_user, similarity_vec, user_emb_w, item_emb_w)
    r = run_bass_kernel_spmd(nc, in_maps, core_ids=list(range(NCORES)),
                             trace=_trace)
    out = np.concatenate([r.results[k]["out"] for k in range(NCORES)], axis=0)
    _CACHE["last_result"] = r
    return out
